# revision 52
# baseline (speedup 1.0000x reference)
"""AliasFreeConv Trainium2 kernel.

Data-parallel over batch: 8 samples -> 8 NeuronCores, no collectives.
Per core:
  style modulation (PE matvec) scales x per-ci; demod (computed from
  sum-of-squares of conv_w overlapped with the conv) and act bias are
  applied per-co on the conv output epilogue ->
  per-sample 3x3 VALID conv as 36 shifted bf16 matmuls per w-column-pair ->
  separable FIR up/down resampling as dense-matrix matmuls (U: 62->128,
  D: 128->64) with DRAM corner-turns between the up stages and between
  the down stages; Prelu(sqrt2*x, 0.2) on ACT.

All heavy matmuls in bf16 (fp32 PSUM accumulation); intermediates held in
bf16 to halve HBM traffic. The paired FIR matmuls use PE row/col tiling
(lo half on partitions 0-63, hi on 64-127) so each pair runs concurrently
on the 128x128 array.
"""
import math
import os
import numpy as np
from contextlib import ExitStack

import ml_dtypes

import concourse.bass as bass
import concourse.bacc as bacc
import concourse.tile as tile
from concourse import mybir
from concourse.bass_utils import run_bass_kernel_spmd

F32 = mybir.dt.float32
BF16 = mybir.dt.bfloat16
AF = mybir.ActivationFunctionType
ALU = mybir.AluOpType

B, CI, CO, H, W = 8, 512, 512, 64, 64
KS, TAPS, UP = 3, 12, 2
HO = H - 2                      # 62 valid conv outputs per axis
STYLE = 512
XPAD = 66 * 64                  # x tile free size: 64 cols x 66 rows (2 zero rows)
NB = HO // 2                    # 31 w-column-pair blocks
K9 = KS * KS
CQ = 256                        # co half width
LIN_SCALE = 1.0 / math.sqrt(STYLE)
WSCALE = 1.0 / math.sqrt(CI * KS * KS)
EPS = 1e-8
SQRT2 = math.sqrt(2.0)

_CACHE: dict = {}


def _build_nc(reps=1, variant=None):
    # variant: None = full kernel; "p1" = phase 1 only; "noc" = single conv
    # matmul per block; "p2" = phase 2 only; "p2g*" = phase-2 g-loop only
    # (p2gd: prelu on DVE, p2gp: no prelu, p2gs: no slab copies, p2gn: no
    # corner DMA) -- dev-only timing bisection knobs
    nc = bacc.Bacc()

    xt_d = nc.declare_dram_parameter("xt", [4, 128, XPAD], BF16, isOutput=False)
    stylec_d = nc.declare_dram_parameter("stylec", [4, 128, 1], F32, isOutput=False)
    modw_d = nc.declare_dram_parameter("modw", [4, 128, 512], F32, isOutput=False)
    modb_d = nc.declare_dram_parameter("modb", [128, 4], F32, isOutput=False)
    cw_d = nc.declare_dram_parameter("cw", [4, 128, K9 * CO], BF16, isOutput=False)
    actb_d = nc.declare_dram_parameter("actb", [1, CO], F32, isOutput=False)
    ulo_d = nc.declare_dram_parameter("ulo", [128, 128], BF16, isOutput=False)
    uhi_d = nc.declare_dram_parameter("uhi", [128, 128], BF16, isOutput=False)
    dmat_d = nc.declare_dram_parameter("dmat", [128, 64], BF16, isOutput=False)
    out_d = nc.declare_dram_parameter("o", [64, 64, CO], BF16, isOutput=True)

    u1_d = nc.dram_tensor("u1scratch", [HO, 128, CO], BF16)
    # corner-turned activations, split by hu half so the H-down lo-half
    # loads can start while the hi-half slabs are still being produced.
    # Layout [wd, hu, co]: the slab writes are contiguous 8 KiB runs and the
    # transposed 1 KiB-run access lands on the (cheaper) read side.
    d1_d = [nc.dram_tensor(f"d1scratch{h}", [64, 64, CO], BF16)  # [wd, hu, co]
            for h in range(2)]

    with ExitStack() as ctx:
        tc = ctx.enter_context(tile.TileContext(nc))
        if reps > 1:
            ctx.enter_context(tc.For_i(0, reps, 1))
        pp = ctx.enter_context(tc.tile_pool(name="persist", bufs=1))

        ulo_t = pp.tile([128, 128], BF16)
        uhi_t = pp.tile([128, 128], BF16)
        dmat_t = pp.tile([128, 64], BF16)
        s_sc = pp.tile([128, 4], F32)
        demod_b = pp.tile([128, CO], F32)
        bias_b = pp.tile([128, CO], BF16)
        nc.sync.dma_start(out=ulo_t, in_=ulo_d[:, :])
        nc.sync.dma_start(out=uhi_t, in_=uhi_d[:, :])
        nc.sync.dma_start(out=dmat_t, in_=dmat_d[:, :])

        with tc.tile_pool(name="xw", bufs=1) as xw, \
             tc.tile_pool(name="cpool", bufs=5) as cpool, \
             tc.tile_pool(name="upool", bufs=6) as upool, \
             tc.tile_pool(name="ps1", bufs=4, space="PSUM") as ps1, \
             tc.tile_pool(name="ps2", bufs=4, space="PSUM") as ps2:
            # ---- phase 1 resident tensors: x and conv weights ----
            xts = [xw.tile([128, XPAD], BF16, tag=f"x{t}", name=f"x{t}") for t in range(4)]
            cws = [xw.tile([128, K9 * CO], BF16, tag=f"w{t}", name=f"w{t}") for t in range(4)]

            cts = {}

            def emit_conv(j):
                # x is stored column-major [ci, w, h]: lhsT for (j, kh, kw)
                # is one contiguous 128-run covering w-columns {2j+kw,
                # 2j+kw+1}. Column-boundary bleed rows land on junk
                # partitions 62/63 and 126/127, which the H-up lhsT slices
                # never read.
                pc = ps1.tile([128, CO], F32, tag="conv")
                n_last = 0 if variant == "noc" else 35
                n_mm = 0
                for t in range(4):
                    for kh in range(KS):
                        for kw in range(KS):
                            rhs = cws[t][:, (kh * 3 + kw) * CO:
                                         (kh * 3 + kw + 1) * CO]
                            base = (2 * j + kw) * 64 + kh
                            lhs = xts[t][:, base:base + 128]
                            if n_mm <= n_last:
                                nc.tensor.matmul(pc, lhs, rhs,
                                                 start=(n_mm == 0),
                                                 stop=(n_mm == n_last))
                            n_mm += 1
                # epilogue: demod (per-co) then act bias, cast to bf16
                ct = cpool.tile([128, CO], BF16, tag="c")
                nc.vector.tensor_mul(ct, pc, demod_b)
                nc.vector.tensor_add(ct, ct, bias_b)
                cts[j] = ct

            def emit_hup(j):
                # H-up pair: lo (w=2j, ct rows 0:62) and hi (w=2j+1,
                # rows 64:126) run as concurrent row-tiles. Emitted one
                # block behind the conv so the PE queue never stalls on
                # the DVE epilogue.
                ct = cts.pop(j)
                pu0 = ps2.tile([128, CO], F32, tag="uh")
                pu1 = ps2.tile([128, CO], F32, tag="uh")
                nc.tensor.matmul(pu0, ulo_t[0:62, :], ct[0:62, :],
                                 start=True, stop=True)
                nc.tensor.matmul(pu1, uhi_t[64:126, :], ct[64:126, :],
                                 start=True, stop=True)
                for dlt, pu in ((0, pu0), (1, pu1)):
                    ut = upool.tile([128, CO], BF16, tag="u1")
                    nc.scalar.copy(out=ut, in_=pu)
                    nc.scalar.dma_start(out=u1_d[2 * j + dlt], in_=ut)

            # ---- prologue: style modulation (scales x), demod (per-co).
            # Emitted before the conv so each engine's queue keeps the demod
            # chain ahead of the first conv epilogue; the chain itself is
            # kept short (bf16 squares/reduces) so conv starts early.
            with tc.tile_pool(name="pre", bufs=1) as pre:
                modw_t = [pre.tile([128, 512], F32, tag=f"mw{t}", name=f"mw{t}") for t in range(4)]
                styl_t = [pre.tile([128, 1], F32, tag=f"st{t}", name=f"st{t}") for t in range(4)]
                modb_t = pre.tile([128, 4], F32)
                actb_t = pre.tile([1, CO], F32)
                ones_row = pre.tile([1, 128], F32)
                sq_t = [pre.tile([128, K9 * CO], BF16, tag=f"sq{i}", name=f"sq{i}")
                        for i in range(2)]
                sqk = [pre.tile([128, CO], BF16, tag=f"sqk{t}", name=f"sqk{t}") for t in range(4)]
                sd_row = pre.tile([1, CO], F32)
                demod_row = pre.tile([1, CO], F32)
                s2_t = pre.tile([128, 4], BF16)
                eps_t = pre.tile([1, 1], F32)
                for t in range(4):
                    nc.sync.dma_start(out=styl_t[t], in_=stylec_d[t])
                    nc.sync.dma_start(out=modw_t[t], in_=modw_d[t])
                nc.sync.dma_start(out=modb_t, in_=modb_d[:, :])
                nc.sync.dma_start(out=actb_t, in_=actb_d[:, :])
                nc.vector.memset(ones_row, 1.0)
                nc.vector.memset(eps_t, EPS * (1.0 + EPS) ** 2)
                for t in range(4):
                    nc.sync.dma_start(out=cws[t], in_=cw_d[t])
                for t in range(4):
                    nc.scalar.dma_start(out=xts[t], in_=xt_d[t])

                # s = style @ (mod_w*lin_scale).T; s_sc = wscale*(s + mod_b)
                pt_s = ps1.tile([128, CO], F32, tag="conv")
                for cib in range(4):
                    for dt_ in range(4):
                        nc.tensor.matmul(pt_s[:, cib:cib + 1],
                                         modw_t[dt_][:, cib * 128:(cib + 1) * 128],
                                         styl_t[dt_], start=(dt_ == 0), stop=(dt_ == 3))
                nc.vector.scalar_tensor_tensor(
                    out=s_sc, in0=pt_s[:, 0:4], scalar=WSCALE,
                    in1=modb_t, op0=ALU.mult, op1=ALU.add)

                # x <- x * s_sc (per-ci modulation on the input, not the weights)
                for t in range(4):
                    nc.vector.tensor_scalar_mul(xts[t], xts[t], s_sc[:, t:t + 1])

                # Q_t[ci, co] = sum_k conv_w^2 (bf16, double-buffered so the
                # ACT squares pipeline against the DVE reduces)
                for t in range(4):
                    sq = sq_t[t % 2]
                    nc.scalar.activation(out=sq, in_=cws[t], func=AF.Square)
                    sq3 = bass.AP(tensor=sq.tensor, offset=sq.offset,
                                  ap=[sq.ap[0], [1, CO], [CO, K9]])
                    with nc.allow_low_precision(reason="demod tolerates bf16"):
                        nc.vector.tensor_reduce(out=sqk[t], in_=sq3,
                                                axis=mybir.AxisListType.X,
                                                op=ALU.add)
                nc.scalar.activation(out=s2_t, in_=s_sc, func=AF.Square)

                # A[co] = sum_ci s_sc[ci]^2 * Q[ci, co]; demod = rsqrt(A+eps)
                pa = ps1.tile([128, CO], F32, tag="conv")
                for t in range(4):
                    nc.tensor.matmul(pa[0:1, :], s2_t[:, t:t + 1], sqk[t],
                                     start=(t == 0), stop=(t == 3))
                c2 = (1.0 + EPS) ** 2
                nc.scalar.activation(out=sd_row, in_=pa[0:1, :], func=AF.Sqrt,
                                     scale=c2, bias=eps_t)
                nc.vector.reciprocal(demod_row, sd_row)

                # broadcast demod and act_b across partitions via rank-1 matmuls
                pb = ps2.tile([128, CO], F32, tag="uh")
                nc.tensor.matmul(pb, ones_row, demod_row, start=True, stop=True)
                nc.vector.tensor_copy(out=demod_b, in_=pb)
                pb2 = ps2.tile([128, CO], F32, tag="uh")
                nc.tensor.matmul(pb2, ones_row, actb_t, start=True, stop=True)
                nc.vector.tensor_copy(out=bias_b, in_=pb2)

            # ---- phase 1: conv + H-up, streamed to DRAM per w column;
            # H-up runs one block behind the conv (software pipeline) ----
            nb = 0 if (variant or "").startswith("p2") else NB
            for j in range(nb):
                emit_conv(j)
                if j > 0:
                    emit_hup(j - 1)
            if nb:
                emit_hup(nb - 1)

        # ---- phase 2: W-up -> act -> W-down -> corner turn -> H-down, both
        # co halves together so every DMA moves 1 KiB-contiguous chunks.
        # Down matmuls (M=64) run as concurrent col-tile pairs (co half m on
        # psum bank m, partition halves = hu halves); all PSUM drains are
        # full-width [128, 512]. The hu corner turn bounces through DRAM per
        # 16-hu slab, into d1lo (g 0-3) / d1hi (g 4-7) so H-down's e-loads
        # for the hu-lo half overlap the hu-hi half's production.
        with tc.tile_pool(name="u2p", bufs=5) as u2p, \
             tc.tile_pool(name="apool", bufs=6) as apool, \
             tc.tile_pool(name="d1p", bufs=3) as d1p, \
             tc.tile_pool(name="psw", bufs=3, space="PSUM") as psw, \
             tc.tile_pool(name="psd", bufs=2, space="PSUM") as psd:
            for g in range(8 if variant != "p1" else 0):
                u2 = u2p.tile([128, 8 * CO], BF16, tag="u2")
                u2v = u2.rearrange("p (a c) -> p a c", a=8)
                nc.sync.dma_start(out=u2v[0:62],
                                  in_=u1_d[:, 16 * g:16 * g + 8, :])
                nc.sync.dma_start(out=u2v[64:126],
                                  in_=u1_d[:, 16 * g + 8:16 * g + 16, :])
                # slabF[p=(wd, hu-half) | f=(hu_in_half 8, co 512)]
                slabF = d1p.tile([128, 8 * CO], BF16, tag="slab")
                acts = {}

                def emit_wup(a):
                    # single-hu full-co slices: every AP stays contiguous
                    psL = psw.tile([128, 1024], F32, tag="pw")
                    psH = psw.tile([128, 1024], F32, tag="pw")
                    for s in range(2):
                        hl = (2 * a + s) * 512
                        nc.tensor.matmul(psL[:, 512 * s:512 * s + 512],
                                         ulo_t[0:62, :],
                                         u2[0:62, hl:hl + 512],
                                         start=True, stop=True)
                        nc.tensor.matmul(psH[:, 512 * s:512 * s + 512],
                                         uhi_t[64:126, :],
                                         u2[64:126, hl:hl + 512],
                                         start=True, stop=True)
                    a_L = apool.tile([128, 1024], BF16, tag="A")
                    a_H = apool.tile([128, 1024], BF16, tag="A")
                    # ACT is the g-loop bottleneck (~2.2us per [128,1024]
                    # psum prelu): offload the H unit of odd a-blocks to the
                    # DVE as a 2-op prelu, balancing the two drain engines
                    for ps_p, a_p, on_dve in ((psL, a_L, False),
                                              (psH, a_H, a % 2 == 1)):
                        if on_dve:
                            tmp = apool.tile([128, 1024], BF16, tag="pt")
                            nc.vector.tensor_scalar(
                                out=tmp, in0=ps_p, scalar1=0.0,
                                scalar2=SQRT2 * 0.8, op0=ALU.max,
                                op1=ALU.mult)
                            nc.vector.scalar_tensor_tensor(
                                out=a_p, in0=ps_p, scalar=SQRT2 * 0.2,
                                in1=tmp, op0=ALU.mult, op1=ALU.add)
                        else:
                            nc.scalar.activation(out=a_p, in_=ps_p,
                                                 func=AF.Prelu,
                                                 scale=SQRT2, alpha=0.2)
                    acts[a] = (a_L, a_H)

                def emit_wdown(a):
                    a_L, a_H = acts.pop(a)
                    for s in range(2):
                        pdw = psd.tile([128, 512], F32, tag="pdw")
                        nc.tensor.matmul(pdw[0:64, :],
                                         dmat_t, a_L[:, 512 * s:512 * s + 512],
                                         start=True, stop=True,
                                         tile_position=(0, 0))
                        nc.tensor.matmul(pdw[64:128, :],
                                         dmat_t, a_H[:, 512 * s:512 * s + 512],
                                         start=True, stop=True,
                                         tile_position=(0, 64))
                        if variant != "p2gs":
                            nc.vector.tensor_copy(
                                out=slabF[:, (2 * a + s) * 512:(2 * a + s + 1) * 512],
                                in_=pdw)

                # W-down trails W-up by one block so the PE queue never
                # stalls on the Prelu drain
                for a in range(4):
                    emit_wup(a)
                    if a > 0:
                        emit_wdown(a - 1)
                emit_wdown(3)
                # corner turn for this 16-hu slab; overlaps next g.
                # writes alternate between the two HWDGE queues
                tgt = d1_d[g // 4]
                r0 = 16 * (g % 4)
                sv = slabF.rearrange("p (n c) -> p n c", c=CO)
                if variant not in ("p2gn", "p2gs"):
                    nc.scalar.dma_start(out=tgt[:, r0:r0 + 8, :], in_=sv[0:64])
                    nc.scalar.dma_start(out=tgt[:, r0 + 8:r0 + 16, :],
                                        in_=sv[64:128])

        # H-down, streamed per 8-wd chunk; hu-lo/hi contributions in separate
        # psum tiles (row-tiled K=64 matmuls), summed by the DVE drain
        with tc.tile_pool(name="epool", bufs=1) as epool, \
             tc.tile_pool(name="opool", bufs=6) as opool, \
             tc.tile_pool(name="psh", bufs=4, space="PSUM") as psh:
            # all hu-lo chunk loads first: they only depend on d1lo (written
            # by g 0-3) so they prefetch while g 4-7 still compute; the hi
            # loads gate on the last corner turn
            e_ts = []
            for cch in range(0 if (variant or "").startswith("p2g") or variant == "p1" else 8):
                e_t = epool.tile([128, 8 * CO], BF16, tag=f"e{cch}",
                                 name=f"e{cch}")
                e_ts.append(e_t)
                with nc.allow_non_contiguous_dma(reason="corner turn read"):
                    nc.sync.dma_start(
                        out=e_t[0:64, :],
                        in_=d1_d[0][8 * cch:8 * cch + 8, :, :].transpose([1, 0, 2]))
            for cch in range(len(e_ts)):
                e_t = e_ts[cch]
                with nc.allow_non_contiguous_dma(reason="corner turn read"):
                    nc.sync.dma_start(
                        out=e_t[64:128, :],
                        in_=d1_d[1][8 * cch:8 * cch + 8, :, :].transpose([1, 0, 2]))
                for p in range(2):
                    # 4 wd columns per wide psum tile: partitions (hd, wd
                    # pair-sel), banks = wd within pair; one full-width
                    # drain, 4 KiB-run output writes
                    pdh = psh.tile([128, 1024], F32, tag="pdh")
                    for s in range(4):
                        wl = (4 * p + s) * 512
                        nc.tensor.matmul(
                            pdh[64 * (s // 2):64 * (s // 2) + 64,
                                512 * (s % 2):512 * (s % 2) + 512],
                            dmat_t, e_t[:, wl:wl + 512],
                            start=True, stop=True,
                            tile_position=(0, 64 * (s // 2)))
                    ot = opool.tile([128, 1024], BF16, tag="o")
                    # alternate the psum drain between ACT and DVE
                    if p % 2 == 0:
                        nc.scalar.copy(out=ot, in_=pdh)
                    else:
                        nc.vector.tensor_copy(out=ot, in_=pdh)
                    wd0 = 8 * cch + 4 * p
                    otv = ot.rearrange("p (w c) -> p w c", c=CO)
                    eng = nc.scalar if p % 2 == 0 else nc.sync
                    eng.dma_start(out=out_d[:, wd0:wd0 + 2, :],
                                  in_=otv[0:64])
                    eng.dma_start(out=out_d[:, wd0 + 2:wd0 + 4, :],
                                  in_=otv[64:128])

    nc.compile()
    return nc


def _host_prep(x, style, mod_w, mod_b, conv_w, act_b, up_filter, down_filter):
    x = np.ascontiguousarray(x, np.float32)
    style = np.asarray(style, np.float32)
    mod_w = np.asarray(mod_w, np.float32)
    mod_b = np.asarray(mod_b, np.float32)
    conv_w = np.asarray(conv_w, np.float32)
    act_b = np.asarray(act_b, np.float32)
    up_filter = np.asarray(up_filter, np.float64)
    down_filter = np.asarray(down_filter, np.float64)

    # FIR matrices (see upfirdn2d in the reference):
    #   up:   y[o] = sum_i fu[o + 3 - 2i] x[i],   fu = up_filter * 2
    #   down: y[o] = sum_u df[2o + 6 - u] x[u]
    fu = up_filter * UP
    U = np.zeros((HO, 2 * H), np.float32)
    for i in range(HO):
        for o in range(2 * H):
            t = o + 3 - 2 * i
            if 0 <= t < TAPS:
                U[i, o] = fu[t]
    D = np.zeros((2 * H, H), np.float32)
    for u in range(2 * H):
        for o in range(H):
            t = 2 * o + 6 - u
            if 0 <= t < TAPS:
                D[u, o] = down_filter[t]
    bf = ml_dtypes.bfloat16
    ulo = np.zeros((128, 128), np.float32)
    uhi = np.zeros((128, 128), np.float32)
    ulo[0:HO, :] = U
    uhi[64:64 + HO, :] = U
    ulo = ulo.astype(bf)
    uhi = uhi.astype(bf)
    dmat = np.ascontiguousarray(D).astype(bf)

    modw_host = np.ascontiguousarray(
        (mod_w * LIN_SCALE).T.reshape(4, 128, 512), np.float32)
    modb_host = np.ascontiguousarray(
        (WSCALE * mod_b).reshape(4, 128).T, np.float32)
    cw_host = np.ascontiguousarray(
        conv_w.transpose(1, 2, 3, 0).reshape(4, 128, K9 * CO)).astype(bf)
    actb_host = np.ascontiguousarray(act_b.reshape(1, CO), np.float32)

    # column-major x: [ci, w, h], flat + 128 zero pad at the end
    xp = np.zeros((B, 4, 128, XPAD), np.float32)
    xcm = x.reshape(B, 4, 128, 64, 64).transpose(0, 1, 2, 4, 3)  # [.., w, h]
    xp[:, :, :, 0:64 * 64] = xcm.reshape(B, 4, 128, 64 * 64)
    xp = xp.astype(bf)
    stylec = np.ascontiguousarray(style.reshape(B, 4, 128, 1), np.float32)

    shared = {
        "modw": modw_host, "modb": modb_host, "cw": cw_host,
        "actb": actb_host, "ulo": ulo, "uhi": uhi, "dmat": dmat,
    }
    in_maps = []
    for b in range(B):
        im = dict(shared)
        im["xt"] = np.ascontiguousarray(xp[b])
        im["stylec"] = stylec[b]
        in_maps.append(im)
    return in_maps


def kernel(**inputs):
    _install_neff_cache()
    if "nc" not in _CACHE:
        _CACHE["nc"] = _build_nc()
    nc = _CACHE["nc"]
    in_maps = _host_prep(**inputs)
    trace = os.environ.get("AFC_TRACE", "0") == "1"
    res = run_bass_kernel_spmd(nc, in_maps, list(range(B)), trace=trace)
    _CACHE["last_result"] = res
    out = np.stack([r["o"].transpose(2, 0, 1) for r in res.results])
    return np.ascontiguousarray(out, np.float32)


def _install_neff_cache():
    """Disk-cache walrus compiles by BIR hash (compile is ~10 min)."""
    import hashlib
    import shutil as _sh
    from concourse import bass_utils as _bu
    from concourse import bass2jax as _bj
    if getattr(_bu, "_afc_cache_installed", False):
        return
    orig = _bu.compile_bir_kernel
    cache_dir = "/tmp/afc_neff_cache"
    os.makedirs(cache_dir, exist_ok=True)

    def cached(bir_json, tmpdir, neff_name="file.neff"):
        data = bir_json if isinstance(bir_json, bytes) else bir_json.encode()
        h = hashlib.sha256(data).hexdigest()[:24]
        cpath = os.path.join(cache_dir, h + ".neff")
        dst = os.path.join(tmpdir, neff_name)
        if os.path.exists(cpath):
            _sh.copy(cpath, dst)
            return dst
        p = orig(bir_json, tmpdir, neff_name)
        try:
            _sh.copy(p, cpath)
        except OSError:
            pass
        return p

    _bu.compile_bir_kernel = cached
    _bj.compile_bir_kernel = cached
    _bu._afc_cache_installed = True


def _make_runner(nc, in_maps, k=1):
    """Build a reusable jitted shard_map callable over 8 cores with
    device-resident inputs (mirrors bass2jax.run_bass_via_pjrt). With k>1
    the NEFF executes k times per dispatch, chained through the donated
    output operands so XLA cannot CSE or parallelize the calls."""
    import jax
    from jax.experimental.shard_map import shard_map
    from jax.sharding import Mesh, NamedSharding, PartitionSpec
    from concourse import bass2jax

    bass2jax.install_neuronx_cc_hook()
    partition_name = nc.partition_id_tensor.name if nc.partition_id_tensor else None
    in_names, out_names, out_avals, zero_outs = [], [], [], []
    for alloc in nc.m.functions[0].allocations:
        if not isinstance(alloc, mybir.MemoryLocationSet):
            continue
        name = alloc.memorylocations[0].name
        if alloc.kind == "ExternalInput":
            if name != partition_name:
                in_names.append(name)
        elif alloc.kind == "ExternalOutput":
            out_names.append(name)
            shape = tuple(alloc.tensor_shape)
            dtype = mybir.dt.np(alloc.dtype)
            out_avals.append(jax.core.ShapedArray(shape, dtype))
            zero_outs.append(np.zeros(shape, dtype))
    n_params = len(in_names)
    all_names = list(in_names) + out_names
    if partition_name is not None:
        all_names.append(partition_name)

    def _body(*args):
        ins = list(args[:n_params])
        outs = list(args[n_params:])
        for _ in range(k):
            operands = ins + outs
            if partition_name is not None:
                operands.append(bass2jax.partition_id_tensor())
            outs = list(bass2jax._bass_exec_p.bind(
                *operands, out_avals=tuple(out_avals),
                in_names=tuple(all_names), out_names=tuple(out_names),
                lowering_input_output_aliases=(), sim_require_finite=True,
                sim_require_nnan=True, nc=nc))
        return tuple(outs)

    n = len(in_maps)
    devices = jax.devices()[:n]
    mesh = Mesh(np.asarray(devices), ("core",))
    nin = n_params + len(out_names)
    f = jax.jit(shard_map(_body, mesh=mesh,
                          in_specs=(PartitionSpec("core"),) * nin,
                          out_specs=(PartitionSpec("core"),) * len(out_names),
                          check_rep=False), keep_unused=True)
    sh = NamedSharding(mesh, PartitionSpec("core"))
    args = [jax.device_put(
        np.concatenate([np.asarray(m[nm]) for m in in_maps], axis=0), sh)
        for nm in in_names]
    args += [jax.device_put(
        np.zeros((n * z.shape[0], *z.shape[1:]), z.dtype), sh)
        for z in zero_outs]
    return f, args


def _time_runner(f, args, iters):
    import time as _time
    for _ in range(2):
        jax.block_until_ready(f(*args))
    best = float("inf")
    for _ in range(iters):
        t0 = _time.perf_counter()
        jax.block_until_ready(f(*args))
        best = min(best, _time.perf_counter() - t0)
    return best


def time_kernel(iters=6, k1=8, k2=264, k=None, **inputs):
    # `k` accepted for backward compatibility with the original
    # 1x-vs-(1+k)x signature; the paired-difference method ignores it.
    """Per-execution time via in-kernel For_i repeat loops: the pipeline
    runs k1x and k2x per dispatch; the difference isolates device time from
    the ~80 ms axon dispatch overhead. Measurements are interleaved in
    (k1, k2) pairs and the median pair-difference is used, so slow drift
    (thermal/clock state) cancels. Returns ns."""
    global jax
    import jax
    import time as _time
    _install_neff_cache()
    in_maps = _host_prep(**inputs)
    f1, args = _make_runner(_build_nc(reps=k1), in_maps)
    f2, _ = _make_runner(_build_nc(reps=k2), in_maps)
    for f in (f1, f2):
        for _ in range(2):
            jax.block_until_ready(f(*args))
    diffs = []
    for _ in range(iters):
        t0 = _time.perf_counter()
        jax.block_until_ready(f1(*args))
        t1 = _time.perf_counter()
        jax.block_until_ready(f2(*args))
        t2 = _time.perf_counter()
        diffs.append(((t2 - t1) - (t1 - t0)) / (k2 - k1))
    diffs.sort()
    med = diffs[len(diffs) // 2]
    print("pair diffs (us):", " ".join(f"{d*1e6:.0f}" for d in diffs))
    return med * 1e9
